# revision 1
# baseline (speedup 1.0000x reference)
"""LpAlignEntropyLoss Trainium2 kernel (8 NeuronCores, SPMD).

loss = mean_i ||v0_i - v1_i||_2
     + 0.5*(mean_i lme0_i + mean_i lme1_i) - log(N-1)
where lme_i = log(sum_{j!=i} exp(-||z_i - z_j||_2)) per view.

Strategy (row-sharded, flash-style):
  Each core owns 1024 rows of each view. It computes its [1024, 8192]
  pairwise-distance tile against the full (device-transposed, bf16) z via
  TensorE GEMMs (d2 = sq_i + sq_j - 2*z_i.z_j accumulated in PSUM with the
  sq_j term folded in as a K=1 matmul row and sq_i applied as the ScalarE
  activation bias), ScalarE Sqrt -> d (bf16), ScalarE Exp with fused
  row-sum accumulation -> S_i, Ln -> lme_i.  The diagonal is masked by
  accumulating +BIG onto it with a per-core identity-select matmul.
  Host combines the 8 cores' per-row partials into the scalar loss.
"""

import sys

for _p in ("/opt/trn_rl_repo",):
    if _p not in sys.path:
        sys.path.insert(0, _p)

import math

import ml_dtypes
import numpy as np

import concourse.bass as bass
from concourse import bacc
import concourse.mybir as mybir
import concourse.tile as tile
from concourse.bass import ds, ts
from concourse.tile import add_dep_helper

F32 = mybir.dt.float32
BF16 = mybir.dt.bfloat16
AF = mybir.ActivationFunctionType
ALU = mybir.AluOpType

N = 8192          # rows per view
K = 256           # features
NCORES = 8
R = N // NCORES   # rows per core = 1024
RT = R // 128     # row tiles per core = 8
CCH = 1024        # column chunk
NCH = N // CCH    # 8 chunks
HALF_RT = 4       # row-tiles per ACT table window
BIG = 30000.0     # diagonal d2 offset -> exp(-sqrt(BIG)) == 0 in f32


def build_nc():
    nc = bacc.Bacc()

    v_in = [
        nc.declare_dram_parameter("v0", [N, K], F32, isOutput=False),
        nc.declare_dram_parameter("v1", [N, K], F32, isOutput=False),
    ]
    own_in = [
        nc.declare_dram_parameter("own0", [R, K], F32, isOutput=False),
        nc.declare_dram_parameter("own1", [R, K], F32, isOutput=False),
    ]
    eye_in = nc.declare_dram_parameter("eye", [128, 128], BF16, isOutput=False)
    dsel_in = nc.declare_dram_parameter("diagsel", [128, NCH * 128], BF16, isOutput=False)
    out_ext = nc.declare_dram_parameter("out", [4, R], F32, isOutput=True)

    sq_scr = [
        nc.dram_tensor("sqscr0", [1, R], F32, kind="Internal"),
        nc.dram_tensor("sqscr1", [1, R], F32, kind="Internal"),
    ]

    act_windows = []  # list of lists of BassInstruction (ACT) to order table use

    with tile.TileContext(nc) as tc:
        with (
            tc.tile_pool(name="consts", bufs=1) as consts,
            tc.tile_pool(name="persist", bufs=1) as persist,
            tc.tile_pool(name="zt", bufs=1) as ztp,
            tc.tile_pool(name="dpool", bufs=4) as dpool,
            tc.tile_pool(name="nat", bufs=4) as natp,
            tc.tile_pool(name="chunk", bufs=4) as chp,
            tc.tile_pool(name="mmps", bufs=3, space="PSUM") as mmps,
            tc.tile_pool(name="auxps", bufs=2, space="PSUM") as auxps,
            tc.tile_pool(name="dram", bufs=2, space="DRAM") as dramp,
        ):
            # ---------------- constants ----------------
            ones_col = consts.tile([128, 1], BF16, name="ones_col")
            nc.vector.memset(ones_col, 1.0)
            ones_row = consts.tile([1, 128], BF16, name="ones_row")
            nc.vector.memset(ones_row, 1.0)
            eye_sb = consts.tile([128, 128], BF16, name="eye_sb")
            nc.sync.dma_start(out=eye_sb, in_=eye_in[:, :])
            dsel_sb = consts.tile([128, NCH * 128], BF16, name="dsel_sb")
            nc.sync.dma_start(out=dsel_sb, in_=dsel_in[:, :])

            # ---------------- own-rows prep (both views) ----------------
            qt = {}       # qt[(v, kt)] : [128, R] bf16  (own rows, transposed)
            sq_own = {}   # [1, R] f32
            bias_sb = {}  # [128, RT] f32 = sq_i + 256
            for v in (0, 1):
                for kt in (0, 1):
                    qt[(v, kt)] = persist.tile([128, R], BF16, name=f"qt{v}{kt}")
                scrq = dramp.tile([R, K], BF16, name="scrq", tag="scrq")
                for t in range(RT):
                    o32 = natp.tile([128, K], F32, name="o32", tag="nat32")
                    nc.sync.dma_start(out=o32, in_=own_in[v][ts(t, 128), :])
                    o16 = natp.tile([128, K], BF16, name="o16", tag="nat16")
                    nc.vector.tensor_copy(o16, o32)
                    nc.sync.dma_start(out=scrq[ts(t, 128), :], in_=o16)
                for kt in (0, 1):
                    for c in range(R // 512):
                        nc.sync.dma_start_transpose(
                            qt[(v, kt)][:, ts(c, 512)],
                            scrq[ts(c, 512), ts(kt, 128)],
                        )
                # sq_own via squared-qt column sums
                q2 = {}
                for kt in (0, 1):
                    q2[kt] = chp.tile([128, R], BF16, name="q2", tag="chunk")
                    nc.vector.tensor_mul(q2[kt], qt[(v, kt)], qt[(v, kt)])
                sq_own[v] = persist.tile([1, R], F32, name=f"sq_own{v}")
                for n in range(R // 512):
                    ps = auxps.tile([1, 512], F32, name="sqps", tag="aux")
                    for kt in (0, 1):
                        nc.tensor.matmul(
                            ps, ones_col, q2[kt][:, ts(n, 512)],
                            start=(kt == 0), stop=(kt == 1),
                        )
                    nc.vector.tensor_copy(sq_own[v][:, ts(n, 512)], ps)
                # roundtrip to get [128, RT] per-partition bias layout
                wr = nc.sync.dma_start(out=sq_scr[v][:, :], in_=sq_own[v])
                bias_raw = persist.tile([128, RT], F32, name=f"bias_raw{v}")
                rd = nc.sync.dma_start(
                    out=bias_raw,
                    in_=sq_scr[v].rearrange("o (t p) -> (o p) t", p=128),
                )
                add_dep_helper(rd.ins, wr.ins, True, "sq scratch RAW")
                bias_sb[v] = persist.tile([128, RT], F32, name=f"bias_sb{v}")
                nc.vector.tensor_scalar_add(bias_sb[v], bias_raw, 256.0)

            # ---------------- align term ----------------
            s01 = persist.tile([1, R], F32, name="s01")
            nc.vector.tensor_add(s01, sq_own[0], sq_own[1])
            d2a = persist.tile([1, R], F32, name="d2a")
            qp = {}
            for kt in (0, 1):
                qp[kt] = chp.tile([128, R], BF16, name="qp", tag="chunk")
                nc.vector.tensor_mul(qp[kt], qt[(0, kt)], qt[(1, kt)])
            for n in range(R // 512):
                ps = auxps.tile([1, 512], F32, name="dotps", tag="aux")
                for kt in (0, 1):
                    nc.tensor.matmul(
                        ps, ones_col, qp[kt][:, ts(n, 512)],
                        start=(kt == 0), stop=(kt == 1),
                    )
                nc.vector.scalar_tensor_tensor(
                    out=d2a[:, ts(n, 512)], in0=ps, scalar=-2.0,
                    in1=s01[:, ts(n, 512)], op0=ALU.mult, op1=ALU.add,
                )
            align_norm = persist.tile([1, R], F32, name="align_norm")

            # ---------------- per-view main ----------------
            lme_sb = {}
            for v in (0, 1):
                zt = {}
                for kt in (0, 1):
                    zt[kt] = ztp.tile([128, N], BF16, name=f"zt{kt}", tag=f"zt{kt}")
                scrz = dramp.tile([N, K], BF16, name="scrz", tag="scrz")
                for t in range(N // 128):
                    n32 = natp.tile([128, K], F32, name="n32", tag="nat32")
                    nc.sync.dma_start(out=n32, in_=v_in[v][ts(t, 128), :])
                    n16 = natp.tile([128, K], BF16, name="n16", tag="nat16")
                    nc.vector.tensor_copy(n16, n32)
                    nc.sync.dma_start(out=scrz[ts(t, 128), :], in_=n16)
                for kt in (0, 1):
                    for c in range(N // 512):
                        nc.sync.dma_start_transpose(
                            zt[kt][:, ts(c, 512)],
                            scrz[ts(c, 512), ts(kt, 128)],
                        )
                # sqc row: (256 - sq_j) / 2 as bf16 [1, N]
                sqc = ztp.tile([1, N], BF16, name="sqc", tag="sqc")
                for c in range(NCH):
                    z2 = {}
                    for kt in (0, 1):
                        z2[kt] = chp.tile([128, CCH], BF16, name="z2", tag="chunk")
                        nc.vector.tensor_mul(
                            z2[kt], zt[kt][:, ts(c, CCH)], zt[kt][:, ts(c, CCH)]
                        )
                    for n in range(CCH // 512):
                        ps = auxps.tile([1, 512], F32, name="sqfps", tag="aux")
                        for kt in (0, 1):
                            nc.tensor.matmul(
                                ps, ones_col, z2[kt][:, ts(n, 512)],
                                start=(kt == 0), stop=(kt == 1),
                            )
                        nc.vector.tensor_scalar(
                            out=sqc[:, ds(c * CCH + n * 512, 512)], in0=ps,
                            scalar1=-0.5, scalar2=128.0,
                            op0=ALU.mult, op1=ALU.add,
                        )

                S_sb = persist.tile([128, RT], F32, name=f"S_sb{v}")
                for half in range(RT // HALF_RT):
                    sqrt_w = []
                    exp_w = []
                    d_tiles = {}
                    for rt in range(half * HALF_RT, (half + 1) * HALF_RT):
                        dt_ = dpool.tile([128, N], BF16, name="dt", tag="d")
                        d_tiles[rt] = dt_
                        o = rt * 128  # within-chunk diagonal offset
                        dbank = o // 512
                        for c in range(NCH):
                            ps = mmps.tile([128, CCH], F32, name="mmtile", tag="mm")
                            for n in range(CCH // 512):
                                nc.tensor.matmul(
                                    ps[:, ts(n, 512)], qt[(v, 0)][:, ts(rt, 128)],
                                    zt[0][:, ds(c * CCH + n * 512, 512)],
                                    start=True, stop=False,
                                )
                                nc.tensor.matmul(
                                    ps[:, ts(n, 512)], qt[(v, 1)][:, ts(rt, 128)],
                                    zt[1][:, ds(c * CCH + n * 512, 512)],
                                    start=False, stop=False,
                                )
                                nc.tensor.matmul(
                                    ps[:, ts(n, 512)], ones_row,
                                    sqc[:, ds(c * CCH + n * 512, 512)],
                                    start=False, stop=(n != dbank),
                                    skip_group_check=True,
                                )
                            nc.tensor.matmul(
                                ps[:, ds(o, 128)], dsel_sb[:, ts(c, 128)], eye_sb,
                                start=False, stop=True, skip_group_check=True,
                            )
                            si = nc.scalar.activation(
                                out=dt_[:, ts(c, CCH)], in_=ps, func=AF.Sqrt,
                                bias=bias_sb[v][:, ds(rt, 1)], scale=-2.0,
                            )
                            sqrt_w.append(si)
                        # end chunks for this rt
                    if v == 0 and half == 0:
                        ai = nc.scalar.activation(
                            out=align_norm, in_=d2a, func=AF.Sqrt
                        )
                        sqrt_w.append(ai)
                    for rt in range(half * HALF_RT, (half + 1) * HALF_RT):
                        escr = dpool.tile([128, N], BF16, name="escr", tag="escr", bufs=1)
                        ei = nc.scalar.activation(
                            out=escr, in_=d_tiles[rt], func=AF.Exp, scale=-1.0,
                            accum_out=S_sb[:, ds(rt, 1)],
                        )
                        exp_w.append(ei)
                    if half == RT // HALF_RT - 1:
                        lme_sb[v] = persist.tile([128, RT], F32, name=f"lme{v}")
                        li = nc.scalar.activation(
                            out=lme_sb[v], in_=S_sb, func=AF.Ln
                        )
                        exp_w.append(li)
                    act_windows.append(sqrt_w)
                    act_windows.append(exp_w)

                nc.sync.dma_start(
                    out=out_ext[v].rearrange("(t p) -> p t", p=128),
                    in_=lme_sb[v],
                )
            nc.sync.dma_start(out=out_ext[ds(2, 1)], in_=align_norm)

            # order ACT windows to minimize table switches
            for a, b in zip(act_windows, act_windows[1:]):
                if a and b:
                    add_dep_helper(b[0].ins, a[-1].ins, False, "act window order")

    nc.finalize()
    return nc


_NC = None


def _get_nc():
    global _NC
    if _NC is None:
        _NC = build_nc()
    return _NC


def _in_maps(v0, v1):
    v0 = np.ascontiguousarray(v0, dtype=np.float32)
    v1 = np.ascontiguousarray(v1, dtype=np.float32)
    eye = np.eye(128, dtype=ml_dtypes.bfloat16)
    maps = []
    for i in range(NCORES):
        dsel = np.zeros((128, NCH * 128), dtype=ml_dtypes.bfloat16)
        dsel[:, i * 128:(i + 1) * 128] = (-BIG / 2.0) * np.eye(
            128, dtype=np.float32
        ).astype(ml_dtypes.bfloat16)
        maps.append({
            "v0": v0,
            "v1": v1,
            "own0": v0[i * R:(i + 1) * R],
            "own1": v1[i * R:(i + 1) * R],
            "eye": eye,
            "diagsel": dsel,
        })
    return maps


def _combine(results):
    lme0, lme1, aligns = [], [], []
    for res in results:
        o = res["out"]
        lme0.append(o[0])
        lme1.append(o[1])
        aligns.append(o[2])
    lme0 = np.concatenate(lme0)
    lme1 = np.concatenate(lme1)
    aligns = np.concatenate(aligns)
    entropy = 0.5 * (lme0.mean(dtype=np.float64) + lme1.mean(dtype=np.float64)) \
        - math.log(N - 1)
    loss = aligns.mean(dtype=np.float64) + entropy
    return np.float32(loss)


def run_device(v0, v1, trace=False):
    from concourse.bass_utils import run_bass_kernel_spmd

    nc = _get_nc()
    res = run_bass_kernel_spmd(
        nc, _in_maps(v0, v1), core_ids=list(range(NCORES)), trace=trace
    )
    return res


def kernel(v0, v1):
    res = run_device(v0, v1, trace=False)
    return _combine(res.results)


if __name__ == "__main__":
    rng = np.random.default_rng(0)
    v0 = rng.standard_normal((N, K), dtype=np.float32)
    v1 = rng.standard_normal((N, K), dtype=np.float32)
    print("building...")
    nc = _get_nc()
    print("running...")
    out = kernel(v0, v1)
    print("loss:", out)



# revision 5
# speedup vs baseline: 1.6299x; 1.6299x over previous
"""LpAlignEntropyLoss Trainium2 kernel (8 NeuronCores, SPMD).

loss = mean_i ||v0_i - v1_i||_2                                (align, host)
     + 0.5*sum_views mean_i [ln S_i - ln(N-1)],  S_i = sum_{j!=i} exp(-d_ij)

Symmetric "tournament" scheme (halves the O(N^2) work):
  N=8192 rows = 64 blocks of 128. Core c receives z ROTATED by -1024c rows,
  so its own rows are local rows 0..1023. Local row-tile k (rows 128k..+128)
  computes pairwise distances against local cols [128k, 128k+4224) -- its own
  block plus the 32 blocks "ahead" (mod 64 globally, static locally thanks to
  the rotation). Every unordered pair lands in exactly one tile:
    distance  0 block: full, diag masked (+BIG), row-sums only
    distance 1..31   : row-sums + column-sums
    distance 32 block: computed from BOTH sides, row-sums only
  Device ships per-tile row sums [128] and column partials [3968] (f32);
  host un-rotates, sums partials across cores, takes ln in f64, and adds the
  host-computed align term.

Engines: TensorE does the d2 GEMM (2x K=128) + rank-1 sq_j add + diag mask
+ ones-matmul column sums. ScalarE does Sqrt (PSUM->bf16, per-partition
sq_i bias) and Exp (bf16->bf16 with f32 row-sum accumulation), batched per
view into one sqrt phase + one exp phase to minimize ACT table loads.
Host precomputes sq_j / sqc / bias (O(N*K)).
"""

import sys

for _p in ("/opt/trn_rl_repo",):
    if _p not in sys.path:
        sys.path.insert(0, _p)

import math

import ml_dtypes
import numpy as np

import concourse.bass as bass
from concourse import bacc
import concourse.mybir as mybir
import concourse.tile as tile
from concourse.bass import ds, ts
from concourse.tile import add_dep_helper

F32 = mybir.dt.float32
BF16 = mybir.dt.bfloat16
AF = mybir.ActivationFunctionType
ALU = mybir.AluOpType

N = 8192          # rows per view
K = 256           # features
NCORES = 8
R = N // NCORES   # rows per core = 1024
NT = 8            # row tiles per core
TW = 4224         # cols per tile  (33 blocks of 128)
CSW = 3968        # col-sum region width (31 blocks: skip own + distance-32)
BIG = 30000.0     # diag d2 offset -> exp(-sqrt(BIG)) == 0 in f32
LCH = 1024        # big-load chunk (f32 [128, LCH])
NLCH = 64 * K // LCH  # 4 load chunks per view
TCW = 1024        # dma transpose piece: [TCW, 128] -> [128, TCW]


def build_nc():
    nc = bacc.Bacc()

    v_in = [
        nc.declare_dram_parameter("v0", [N, K], F32, isOutput=False),
        nc.declare_dram_parameter("v1", [N, K], F32, isOutput=False),
    ]
    sqc_in = [
        nc.declare_dram_parameter("sqc0", [1, N], BF16, isOutput=False),
        nc.declare_dram_parameter("sqc1", [1, N], BF16, isOutput=False),
    ]
    bias_in = [
        nc.declare_dram_parameter("bias0", [128, NT], F32, isOutput=False),
        nc.declare_dram_parameter("bias1", [128, NT], F32, isOutput=False),
    ]
    eye_in = nc.declare_dram_parameter("eye", [128, 128], BF16, isOutput=False)
    nbe_in = nc.declare_dram_parameter("negbigeye", [128, 128], BF16, isOutput=False)
    srow_ext = nc.declare_dram_parameter("srow", [128, 2 * NT], F32, isOutput=True)
    colp_ext = nc.declare_dram_parameter("colp", [2 * NT, CSW], F32, isOutput=True)

    with tile.TileContext(nc) as tc:
        with (
            tc.tile_pool(name="consts", bufs=1) as consts,
            tc.tile_pool(name="persist", bufs=1) as persist,
            tc.tile_pool(name="zt", bufs=1) as ztp,
            tc.tile_pool(name="dpool", bufs=8) as dpool,
            tc.tile_pool(name="epool", bufs=2) as epool,
            tc.tile_pool(name="nat", bufs=2) as natp,
            tc.tile_pool(name="mmps", bufs=3, space="PSUM") as mmps,
            tc.tile_pool(name="auxps", bufs=2, space="PSUM") as auxps,
            tc.tile_pool(name="cstage", bufs=1) as cstp,
            tc.tile_pool(name="dram", bufs=2, space="DRAM") as dramp,
        ):
            # ---------------- constants ----------------
            ones_row = consts.tile([1, 128], BF16, name="ones_row")
            nc.vector.memset(ones_row, 1.0)
            ones_col = consts.tile([128, 1], BF16, name="ones_col")
            nc.vector.memset(ones_col, 1.0)
            eye_sb = consts.tile([128, 128], BF16, name="eye_sb")
            nc.sync.dma_start(out=eye_sb, in_=eye_in[:, :])
            nbe_sb = consts.tile([128, 128], BF16, name="nbe_sb")
            nc.sync.dma_start(out=nbe_sb, in_=nbe_in[:, :])
            sqc_sb, bias_sb = {}, {}
            for v in (0, 1):
                sqc_sb[v] = persist.tile([1, N], BF16, name=f"sqc{v}")
                nc.sync.dma_start(out=sqc_sb[v], in_=sqc_in[v][:, :])
                bias_sb[v] = persist.tile([128, NT], F32, name=f"bias{v}")
                nc.sync.dma_start(out=bias_sb[v], in_=bias_in[v][:, :])

            S_sb = persist.tile([128, 2 * NT], F32, name="S_sb")

            # ---------------- z load + transpose (per view) ----------------
            # v [8192, 256] f32 --load--> sbuf [128, 4096] f32 chunks
            # --DVE--> bf16 --store--> scrz [8192, 256] bf16
            # --dma_transpose--> zt[kt] [128, 8192] bf16  (kt = K half)
            def load_view(v):
                zt = {}
                for kt in (0, 1):
                    zt[kt] = ztp.tile([128, N], BF16, name=f"zt{v}{kt}", tag=f"zt{kt}")
                scrz = dramp.tile([N, K], BF16, name="scrz", tag="scrz")
                vsrc = v_in[v].rearrange("(t p) k -> p t k", p=128)
                sdst = scrz.rearrange("(t p) k -> p t k", p=128)
                tch = LCH // K
                for c in range(NLCH):
                    n32 = natp.tile([128, LCH], F32, name="n32", tag="nat32")
                    nc.sync.dma_start(
                        out=n32.rearrange("p (t k) -> p t k", k=K),
                        in_=vsrc[:, ts(c, tch), :],
                    )
                    n16 = natp.tile([128, LCH], BF16, name="n16", tag="nat16")
                    nc.vector.tensor_copy(n16, n32)
                    nc.sync.dma_start(
                        out=sdst[:, ts(c, tch), :],
                        in_=n16.rearrange("p (t k) -> p t k", k=K),
                    )
                # transposed reads: [TCW, 128] -> [128, TCW]
                for kt in (0, 1):
                    eng = nc.scalar if (v == 0 and kt == 1) else nc.sync
                    for c in range(N // TCW):
                        eng.dma_start_transpose(
                            zt[kt][:, ts(c, TCW)],
                            scrz[ts(c, TCW), ts(kt, 128)],
                        )
                return zt

            # ---------------- per-tile GEMM -> psum chunks ----------------
            # psum chunk layout per tile: [1024, 1024, 1024, 1024, 128]
            CHUNKS = [(0, 1024), (1024, 1024), (2048, 1024), (3072, 1024), (4096, 128)]

            def gemm_tile(v, zt, k, act_out):
                """Emit MMs for tile k of view v; call act_out(ps, off, w)
                for each finished psum chunk."""
                lo = 128 * k
                for off, w in CHUNKS:
                    ps = mmps.tile([128, 1024], F32, name="mm", tag="mm")
                    for n0 in range(0, w, 512):
                        nw = min(512, w - n0)
                        cl = lo + off + n0
                        nc.tensor.matmul(
                            ps[:, ds(n0, nw)], zt[0][:, ds(lo, 128)],
                            zt[0][:, ds(cl, nw)], start=True, stop=False,
                        )
                        nc.tensor.matmul(
                            ps[:, ds(n0, nw)], zt[1][:, ds(lo, 128)],
                            zt[1][:, ds(cl, nw)], start=False, stop=False,
                        )
                        last = off + n0 + nw == TW
                        nc.tensor.matmul(
                            ps[:, ds(n0, nw)], ones_row,
                            sqc_sb[v][:, ds(cl, nw)],
                            start=False, stop=(not (off == 0 and n0 == 0)),
                            skip_group_check=True,
                        )
                    if off == 0:
                        nc.tensor.matmul(
                            ps[:, ds(0, 128)], nbe_sb, eye_sb,
                            start=False, stop=True, skip_group_check=True,
                        )
                    act_out(ps, off, w)

            # ---------------- main schedule ----------------
            sqrt_w = {0: [], 1: []}   # ACT sqrt instructions per view
            exp_w = {0: [], 1: []}    # ACT exp instructions per view

            zt_v = {}
            zt_v[0] = load_view(0)
            zt_v[1] = load_view(1)

            e_tiles = {}
            for v in (0, 1):
                d_tiles = {}
                for k in range(NT):
                    dt_ = dpool.tile([128, TW], BF16, name=f"d{v}{k}", tag="d")
                    d_tiles[k] = dt_

                    def p1(ps, off, w, dt_=dt_, v=v, k=k):
                        si = nc.scalar.activation(
                            out=dt_[:, ds(off, w)], in_=ps[:, ds(0, w)],
                            func=AF.Sqrt, bias=bias_sb[v][:, ds(k, 1)],
                            scale=-2.0,
                        )
                        sqrt_w[v].append(si)

                    gemm_tile(v, zt_v[v], k, p1)

                # exp phase for view v; col-sums + v1 GEMM interleave on TensorE
                for k in range(NT):
                    et = epool.tile([128, TW], BF16, name="e", tag="e")
                    ei = nc.scalar.activation(
                        out=et, in_=d_tiles[k], func=AF.Exp, scale=-1.0,
                        accum_out=S_sb[:, ds(v * NT + k, 1)],
                    )
                    exp_w[v].append(ei)
                    e_tiles[(v, k)] = et
                    # column sums over E[:, 128:4096] -> colp row v*NT+k
                    stage = cstp.tile([1, CSW], F32, name="cstage", tag="cst")
                    for n0 in range(0, CSW, 512):
                        nw = min(512, CSW - n0)
                        cs = auxps.tile([1, 512], F32, name="cs", tag="cs")
                        nc.tensor.matmul(
                            cs[:, ds(0, nw)], ones_col, et[:, ds(128 + n0, nw)],
                            start=True, stop=True,
                        )
                        nc.vector.tensor_copy(stage[:, ds(n0, nw)], cs[:, ds(0, nw)])
                    nc.gpsimd.dma_start(
                        out=colp_ext[ds(v * NT + k, 1), :], in_=stage
                    )

            nc.sync.dma_start(out=srow_ext[:, :], in_=S_sb)

            # ACT phase ordering: sqrt(v0) -> exp(v0) -> sqrt(v1) -> exp(v1)
            phases = [sqrt_w[0], exp_w[0], sqrt_w[1], exp_w[1]]
            for a, b in zip(phases, phases[1:]):
                if a and b:
                    add_dep_helper(b[0].ins, a[-1].ins, False, "act phase order")

    nc.finalize()
    return nc


_NC = None


def _get_nc():
    global _NC
    if _NC is None:
        _NC = build_nc()
    return _NC


def _in_maps(v0, v1):
    v0 = np.ascontiguousarray(v0, dtype=np.float32)
    v1 = np.ascontiguousarray(v1, dtype=np.float32)
    eye = np.eye(128, dtype=ml_dtypes.bfloat16)
    nbe = ((-BIG / 2.0) * np.eye(128, dtype=np.float32)).astype(ml_dtypes.bfloat16)
    maps = []
    for c in range(NCORES):
        m = {"eye": eye, "negbigeye": nbe}
        for v, arr in ((0, v0), (1, v1)):
            vrot = np.roll(arr, -R * c, axis=0)
            sq = np.einsum("ij,ij->i", vrot.astype(np.float64), vrot.astype(np.float64))
            m[f"v{v}"] = np.ascontiguousarray(vrot)
            m[f"sqc{v}"] = ((256.0 - sq) / 2.0).astype(ml_dtypes.bfloat16)[None, :]
            m[f"bias{v}"] = np.ascontiguousarray(
                (sq[:R].reshape(NT, 128).T + 256.0).astype(np.float32)
            )
        maps.append(m)
    return maps


def _combine(results, v0, v1):
    ent = 0.0
    for v in (0, 1):
        S = np.zeros(N, np.float64)
        for c, res in enumerate(results):
            srow = res["srow"]          # [128, 16]
            colp = res["colp"]          # [16, 3968]
            Sl = np.zeros(N, np.float64)
            for k in range(NT):
                Sl[128 * k:128 * k + 128] += srow[:, v * NT + k]
                Sl[128 * k + 128:128 * k + 128 + CSW] += colp[v * NT + k]
            S += np.roll(Sl, R * c)
        ent += (np.log(S) - math.log(N - 1)).mean()
    d = v0.astype(np.float64) - v1.astype(np.float64)
    align = np.sqrt((d * d).sum(1)).mean()
    return np.float32(align + ent / 2.0)


def run_device(v0, v1, trace=False):
    from concourse.bass_utils import run_bass_kernel_spmd

    nc = _get_nc()
    res = run_bass_kernel_spmd(
        nc, _in_maps(v0, v1), core_ids=list(range(NCORES)), trace=trace
    )
    return res


def kernel(v0, v1):
    res = run_device(v0, v1, trace=False)
    return _combine(res.results, v0, v1)


if __name__ == "__main__":
    rng = np.random.default_rng(0)
    v0 = rng.standard_normal((N, K), dtype=np.float32)
    v1 = rng.standard_normal((N, K), dtype=np.float32)
    print("building...")
    nc = _get_nc()
    print("running...")
    out = kernel(v0, v1)
    print("loss:", out)


# revision 6
# speedup vs baseline: 1.8835x; 1.1556x over previous
"""LpAlignEntropyLoss Trainium2 kernel (8 NeuronCores, SPMD).

loss = mean_i ||v0_i - v1_i||_2                                (align, host)
     + 0.5*sum_views mean_i [ln S_i - ln(N-1)],  S_i = sum_{j!=i} exp(-d_ij)

Symmetric "tournament" scheme (halves the O(N^2) work):
  N=8192 rows = 64 blocks of 128. Core c receives z ROTATED by -1024c rows,
  so its own rows are local rows 0..1023. Local row-tile k (rows 128k..+128)
  computes pairwise distances against local cols [128k, 128k+4224) -- its own
  block plus the 32 blocks "ahead" (mod 64 globally, static locally thanks to
  the rotation). Every unordered pair lands in exactly one tile:
    distance  0 block: full, diag masked (+BIG), row-sums only
    distance 1..31   : row-sums + column-sums
    distance 32 block: computed from BOTH sides, row-sums only
  Device ships per-tile row sums [128] and column partials [3968] (f32);
  host un-rotates, sums partials across cores, takes ln in f64, and adds the
  host-computed align term.

Engines: TensorE does the d2 GEMM (2x K=128) + rank-1 sq_j add + diag mask
+ ones-matmul column sums. ScalarE does Sqrt (PSUM->bf16, per-partition
sq_i bias) and Exp (bf16->bf16 with f32 row-sum accumulation), batched per
view into one sqrt phase + one exp phase to minimize ACT table loads.
Host precomputes sq_j / sqc / bias (O(N*K)).
"""

import sys

for _p in ("/opt/trn_rl_repo",):
    if _p not in sys.path:
        sys.path.insert(0, _p)

import math

import ml_dtypes
import numpy as np

import concourse.bass as bass
from concourse import bacc
import concourse.mybir as mybir
import concourse.tile as tile
from concourse.bass import ds, ts
from concourse.tile import add_dep_helper

F32 = mybir.dt.float32
BF16 = mybir.dt.bfloat16
AF = mybir.ActivationFunctionType
ALU = mybir.AluOpType

N = 8192          # rows per view
K = 256           # features
NCORES = 8
R = N // NCORES   # rows per core = 1024
NT = 8            # row tiles per core
TW = 4224         # cols per tile  (33 blocks of 128)
CSW = 3968        # col-sum region width (31 blocks: skip own + distance-32)
BIG = 30000.0     # diag d2 offset -> exp(-sqrt(BIG)) == 0 in f32
LCH = 1024        # big-load chunk (f32 [128, LCH])
NLCH = 64 * K // LCH  # 4 load chunks per view
TCW = 1024        # dma transpose piece: [TCW, 128] -> [128, TCW]


def build_nc():
    nc = bacc.Bacc()

    v_in = [
        nc.declare_dram_parameter("v0", [N, K], F32, isOutput=False),
        nc.declare_dram_parameter("v1", [N, K], F32, isOutput=False),
    ]
    sqc_in = [
        nc.declare_dram_parameter("sqc0", [1, N], BF16, isOutput=False),
        nc.declare_dram_parameter("sqc1", [1, N], BF16, isOutput=False),
    ]
    bias_in = [
        nc.declare_dram_parameter("bias0", [128, NT], F32, isOutput=False),
        nc.declare_dram_parameter("bias1", [128, NT], F32, isOutput=False),
    ]
    eye_in = nc.declare_dram_parameter("eye", [128, 128], BF16, isOutput=False)
    nbe_in = nc.declare_dram_parameter("negbigeye", [128, 128], BF16, isOutput=False)
    srow_ext = nc.declare_dram_parameter("srow", [128, 2 * NT], F32, isOutput=True)
    colp_ext = nc.declare_dram_parameter("colp", [2 * NT, CSW], F32, isOutput=True)

    with tile.TileContext(nc) as tc:
        with (
            tc.tile_pool(name="consts", bufs=1) as consts,
            tc.tile_pool(name="persist", bufs=1) as persist,
            tc.tile_pool(name="zt", bufs=2) as ztp,
            tc.tile_pool(name="dpool", bufs=8) as dpool,
            tc.tile_pool(name="epool", bufs=2) as epool,
            tc.tile_pool(name="nat", bufs=2) as natp,
            tc.tile_pool(name="mmps", bufs=3, space="PSUM") as mmps,
            tc.tile_pool(name="auxps", bufs=2, space="PSUM") as auxps,
            tc.tile_pool(name="cstage", bufs=1) as cstp,
            tc.tile_pool(name="sqcp", bufs=1) as sqcp,
            tc.tile_pool(name="dram", bufs=2, space="DRAM") as dramp,
        ):
            # ---------------- constants ----------------
            ones_row = consts.tile([1, 128], BF16, name="ones_row")
            nc.vector.memset(ones_row, 1.0)
            ones_col = consts.tile([128, 1], BF16, name="ones_col")
            nc.vector.memset(ones_col, 1.0)
            eye_sb = consts.tile([128, 128], BF16, name="eye_sb")
            nc.sync.dma_start(out=eye_sb, in_=eye_in[:, :])
            nbe_sb = consts.tile([128, 128], BF16, name="nbe_sb")
            nc.sync.dma_start(out=nbe_sb, in_=nbe_in[:, :])
            sqc_sb, bias_sb = {}, {}
            for v in (0, 1):
                bias_sb[v] = persist.tile([128, NT], F32, name=f"bias{v}")
                nc.sync.dma_start(out=bias_sb[v], in_=bias_in[v][:, :])

            S_sb = persist.tile([128, 2 * NT], F32, name="S_sb")

            # ---------------- z load + transpose (per view) ----------------
            # v [8192, 256] f32 --load--> sbuf [128, 4096] f32 chunks
            # --DVE--> bf16 --store--> scrz [8192, 256] bf16
            # --dma_transpose--> zt[kt] [128, 8192] bf16  (kt = K half)
            def load_view(v):
                zt = {}
                for kt in (0, 1):
                    zt[kt] = ztp.tile([128, N], BF16, name=f"zt{v}{kt}", tag=f"zt{kt}")
                scrz = dramp.tile([N, K], BF16, name="scrz", tag="scrz")
                sqc_sb[v] = sqcp.tile([1, N], BF16, name=f"sqc{v}", tag="sqc")
                nc.sync.dma_start(out=sqc_sb[v], in_=sqc_in[v][:, :])
                vsrc = v_in[v].rearrange("(t p) k -> p t k", p=128)
                sdst = scrz.rearrange("(t p) k -> p t k", p=128)
                tch = LCH // K
                for c in range(NLCH):
                    n32 = natp.tile([128, LCH], F32, name="n32", tag="nat32")
                    nc.gpsimd.dma_start(
                        out=n32.rearrange("p (t k) -> p t k", k=K),
                        in_=vsrc[:, ts(c, tch), :],
                    )
                    n16 = natp.tile([128, LCH], BF16, name="n16", tag="nat16")
                    nc.vector.tensor_copy(n16, n32)
                    nc.sync.dma_start(
                        out=sdst[:, ts(c, tch), :],
                        in_=n16.rearrange("p (t k) -> p t k", k=K),
                    )
                # transposed reads: [TCW, 128] -> [128, TCW], kt-inner so
                # the first GEMM tile unblocks as early as possible
                for c in range(N // TCW):
                    for kt in (0, 1):
                        eng = nc.scalar if (v == 0 and kt == 1) else nc.sync
                        eng.dma_start_transpose(
                            zt[kt][:, ts(c, TCW)],
                            scrz[ts(c, TCW), ts(kt, 128)],
                        )
                return zt

            # ---------------- per-tile GEMM -> psum chunks ----------------
            # psum chunk layout per tile: [1024, 1024, 1024, 1024, 128]
            CHUNKS = [(0, 1024), (1024, 1024), (2048, 1024), (3072, 1024), (4096, 128)]

            def gemm_tile(v, zt, k, act_out):
                """Emit MMs for tile k of view v; call act_out(ps, off, w)
                for each finished psum chunk."""
                lo = 128 * k
                for off, w in CHUNKS:
                    ps = mmps.tile([128, 1024], F32, name="mm", tag="mm")
                    for n0 in range(0, w, 512):
                        nw = min(512, w - n0)
                        cl = lo + off + n0
                        nc.tensor.matmul(
                            ps[:, ds(n0, nw)], zt[0][:, ds(lo, 128)],
                            zt[0][:, ds(cl, nw)], start=True, stop=False,
                        )
                        nc.tensor.matmul(
                            ps[:, ds(n0, nw)], zt[1][:, ds(lo, 128)],
                            zt[1][:, ds(cl, nw)], start=False, stop=False,
                        )
                        last = off + n0 + nw == TW
                        nc.tensor.matmul(
                            ps[:, ds(n0, nw)], ones_row,
                            sqc_sb[v][:, ds(cl, nw)],
                            start=False, stop=(not (off == 0 and n0 == 0)),
                            skip_group_check=True,
                        )
                    if off == 0:
                        nc.tensor.matmul(
                            ps[:, ds(0, 128)], nbe_sb, eye_sb,
                            start=False, stop=True, skip_group_check=True,
                        )
                    act_out(ps, off, w)

            # ---------------- main schedule ----------------
            sqrt_w = {0: [], 1: []}   # ACT sqrt instructions per view
            exp_w = {0: [], 1: []}    # ACT exp instructions per view

            zt_v = {}
            zt_v[0] = load_view(0)
            zt_v[1] = load_view(1)

            e_tiles = {}
            for v in (0, 1):
                d_tiles = {}
                for k in range(NT):
                    dt_ = dpool.tile([128, TW], BF16, name=f"d{v}{k}", tag="d")
                    d_tiles[k] = dt_

                    def p1(ps, off, w, dt_=dt_, v=v, k=k):
                        si = nc.scalar.activation(
                            out=dt_[:, ds(off, w)], in_=ps[:, ds(0, w)],
                            func=AF.Sqrt, bias=bias_sb[v][:, ds(k, 1)],
                            scale=-2.0,
                        )
                        sqrt_w[v].append(si)

                    gemm_tile(v, zt_v[v], k, p1)

                # exp phase for view v; col-sums + v1 GEMM interleave on TensorE
                for k in range(NT):
                    et = epool.tile([128, TW], BF16, name="e", tag="e")
                    ei = nc.scalar.activation(
                        out=et, in_=d_tiles[k], func=AF.Exp, scale=-1.0,
                        accum_out=S_sb[:, ds(v * NT + k, 1)],
                    )
                    exp_w[v].append(ei)
                    e_tiles[(v, k)] = et
                    # column sums over E[:, 128:4096] -> colp row v*NT+k
                    stage = cstp.tile([1, CSW], F32, name="cstage", tag="cst")
                    for n0 in range(0, CSW, 512):
                        nw = min(512, CSW - n0)
                        cs = auxps.tile([1, 512], F32, name="cs", tag="cs")
                        nc.tensor.matmul(
                            cs[:, ds(0, nw)], ones_col, et[:, ds(128 + n0, nw)],
                            start=True, stop=True,
                        )
                        nc.vector.tensor_copy(stage[:, ds(n0, nw)], cs[:, ds(0, nw)])
                    nc.gpsimd.dma_start(
                        out=colp_ext[ds(v * NT + k, 1), :], in_=stage
                    )

            nc.sync.dma_start(out=srow_ext[:, :], in_=S_sb)

            # ACT phase ordering: sqrt(v0) -> exp(v0) -> sqrt(v1) -> exp(v1)
            phases = [sqrt_w[0], exp_w[0], sqrt_w[1], exp_w[1]]
            for a, b in zip(phases, phases[1:]):
                if a and b:
                    add_dep_helper(b[0].ins, a[-1].ins, False, "act phase order")

    nc.finalize()
    return nc


_NC = None


def _get_nc():
    global _NC
    if _NC is None:
        _NC = build_nc()
    return _NC


def _in_maps(v0, v1):
    v0 = np.ascontiguousarray(v0, dtype=np.float32)
    v1 = np.ascontiguousarray(v1, dtype=np.float32)
    eye = np.eye(128, dtype=ml_dtypes.bfloat16)
    nbe = ((-BIG / 2.0) * np.eye(128, dtype=np.float32)).astype(ml_dtypes.bfloat16)
    maps = []
    for c in range(NCORES):
        m = {"eye": eye, "negbigeye": nbe}
        for v, arr in ((0, v0), (1, v1)):
            vrot = np.roll(arr, -R * c, axis=0)
            sq = np.einsum("ij,ij->i", vrot.astype(np.float64), vrot.astype(np.float64))
            m[f"v{v}"] = np.ascontiguousarray(vrot)
            m[f"sqc{v}"] = ((256.0 - sq) / 2.0).astype(ml_dtypes.bfloat16)[None, :]
            m[f"bias{v}"] = np.ascontiguousarray(
                (sq[:R].reshape(NT, 128).T + 256.0).astype(np.float32)
            )
        maps.append(m)
    return maps


def _combine(results, v0, v1):
    ent = 0.0
    for v in (0, 1):
        S = np.zeros(N, np.float64)
        for c, res in enumerate(results):
            srow = res["srow"]          # [128, 16]
            colp = res["colp"]          # [16, 3968]
            Sl = np.zeros(N, np.float64)
            for k in range(NT):
                Sl[128 * k:128 * k + 128] += srow[:, v * NT + k]
                Sl[128 * k + 128:128 * k + 128 + CSW] += colp[v * NT + k]
            S += np.roll(Sl, R * c)
        ent += (np.log(S) - math.log(N - 1)).mean()
    d = v0.astype(np.float64) - v1.astype(np.float64)
    align = np.sqrt((d * d).sum(1)).mean()
    return np.float32(align + ent / 2.0)


def run_device(v0, v1, trace=False):
    from concourse.bass_utils import run_bass_kernel_spmd

    nc = _get_nc()
    res = run_bass_kernel_spmd(
        nc, _in_maps(v0, v1), core_ids=list(range(NCORES)), trace=trace
    )
    return res


def kernel(v0, v1):
    res = run_device(v0, v1, trace=False)
    return _combine(res.results, v0, v1)


if __name__ == "__main__":
    rng = np.random.default_rng(0)
    v0 = rng.standard_normal((N, K), dtype=np.float32)
    v1 = rng.standard_normal((N, K), dtype=np.float32)
    print("building...")
    nc = _get_nc()
    print("running...")
    out = kernel(v0, v1)
    print("loss:", out)
